# revision 41
# baseline (speedup 1.0000x reference)
"""Trainium2 Bass kernel for nn_Cifar10_JointMembership.

Math (closed form of the reference 2-qubit circuit; verified vs reference):
  a = x[b, i0], b_ = x[b, i1]  (gathered pixel pairs, full angles)
  out[b, 2p,   c] = 0.5 + 0.5*cos(theta_c)*cos(a) - 0.5*sin(theta_c)*sin(a)*sin(b_)
  out[b, 2p+1, c] = 0.5 + 0.5*cos(a)*cos(b_)               (same for all c)

Sharding: pure data parallel, batch dim split across 8 NeuronCores
(128 rows per core); theta replicated. Full inputs in, full output out.

Gather strategy: the only per-partition-independent indexed primitive on
TRN2's GPSIMD is `local_scatter` (per-lane scatter through Q7-local RAM at
streaming rate; the SBUF-read gathers `indirect_copy`/`ap_gather` pay a
non-pipelined ~30 cyc per gathered column — the 393 us baseline). A gather
is the inverse of a scatter, so the host re-encodes pair_idx (pure index
bookkeeping, no model data touched) into scatter form, with destination
slots de-interleaved (sigma(j) = (j%2)*460 + j//2) so all downstream
engine ops are stride-1:
  inv[b, pix]  = sigma(first output slot j with pair_idx[b, j] == pix)
  pc[b, j0]    = compaction slot (0..63) for pixels used >= 3 times
  c1cc[b, *]   = fused chain: dst slots of every occurrence of rank >= 1,
                 fed from [d0 | replicated comp] stream positions
Device pipeline per core (128 rows, one row per partition):
  DMA x with SWDGE f32->fp16 cast; scatter xh by inv -> d0; scatter d0 by
  pc -> comp (64 slots); DVE-replicate comp behind d0; one fused scatter
  by c1cc -> dx; acc = d0+dx (disjoint supports, exact). Trig at full
  width in fp16 (ACT 224-cyc and DVE 58-cyc fixed cost per op makes few
  big ops beat many small ones): DVE compare-wrap range reduction into
  [-pi, pi] (ACT Sin LUT is only valid to |x| <= pi), ACT Sin for sin and
  Sin(-|y|+pi/2) for cos, DVE products, per-class tcc on ACT (fp16 2x),
  even columns via DVE scalar_tensor_tensor (stride-20 f32 writes into
  the output block), odd columns via one broadcast ACT per span; HWDGE
  DMA out per span.
fp16 gather + trig bound the error at ~2e-4 rel, well inside the 2e-2
gate.
"""

import os

os.environ.setdefault("BY_DEFAULT_DISABLE_SUBTILE_DEPS", "1")

import numpy as np

import concourse.bass as bass
import concourse.mybir as mybir
from concourse import library_config
from concourse.tile import TileContext as _TileContext

N_CORES = 8
B_FULL = 1024
B = B_FULL // N_CORES  # 128 rows per core
NPIX = 3072
NPAIR = 460
NIDX = 2 * NPAIR  # 920 gathered values per row
NCLS = 10
NCOMP = 64  # compaction slots per row for pixels used >= 3 times
F32 = mybir.dt.float32
F16 = mybir.dt.float16
I16 = mybir.dt.int16
ALU = mybir.AluOpType
PI = float(np.pi)
TWO_PI = float(2 * np.pi)
HALF_PI = float(np.pi / 2)


class TileContext(_TileContext):
    pass


def _legalize_sync_waits(nc):
    """This walrus build allows only ONE sync-wait per non-EventSemaphore
    instruction (and two on EventSemaphore). Tile's add_semaphores can attach
    several. Hoist excess waits onto EventSemaphore instructions inserted
    immediately before the owner on the same engine — semantically identical
    (same engine stream, waits run first)."""
    n_new = 0
    for f in nc.m.functions:
        for bb in f.blocks:
            out = []
            for inst in bb.instructions:
                si = inst.sync_info
                waits = list(si.on_wait) if si is not None and si.on_wait else []
                cap = 2 if inst.opcode == "EventSemaphore" else 1
                if len(waits) > cap:
                    keep, hoist = waits[:cap], waits[cap:]
                    del si.on_wait[:]
                    for w in keep:
                        si.on_wait.append(w)
                    while hoist:
                        chunk, hoist = hoist[:2], hoist[2:]
                        n_new += 1
                        ev = mybir.InstEventSemaphore(
                            name=f"{inst.name}-hw{n_new}",
                            ins=[],
                            outs=[],
                            engine=inst.engine,
                            sync_info=mybir.SyncInfo(on_wait=chunk, on_update=[]),
                        )
                        out.append(ev)
                out.append(inst)
            bb.instructions = out
    return nc


def build_kernel(
    n_repeat=1,
    n_ranks=6,
    n_spans=2,
    parts="stc",
):
    """One NeuronCore's program: 128 batch rows.

    n_ranks: max index multiplicity covered (rank 0 = first occurrence).
    n_spans: output-stage granularity (divides 460).
    n_repeat: re-runs the whole pipeline (identical results) for timing.
    """
    Sin = mybir.ActivationFunctionType.Sin
    Copy = mybir.ActivationFunctionType.Copy
    Abs = mybir.ActivationFunctionType.Abs

    nc = bass.Bass(detect_race_conditions=False)
    xd = nc.dram_tensor("x", [B, NPIX], F32, kind="ExternalInput")
    invd = nc.dram_tensor("inv", [B, NPIX], I16, kind="ExternalInput")
    NCC = max(n_ranks - 2, 0) * NCOMP
    c1d = nc.dram_tensor("c1cc", [B, NIDX + NCC], I16, kind="ExternalInput")
    pcd = nc.dram_tensor("pc", [B, NIDX], I16, kind="ExternalInput")
    td = nc.dram_tensor("theta", [1, NCLS], F32, kind="ExternalInput")
    od = nc.dram_tensor("out", [B, NIDX * NCLS], F32, kind="ExternalOutput")

    assert NPAIR % n_spans == 0

    with TileContext(nc) as tc:
        with (
            tc.tile_pool(name="const", bufs=1) as cpool,
            tc.tile_pool(name="inp", bufs=1) as ipool,
            tc.tile_pool(name="gat", bufs=2) as gpool,
            tc.tile_pool(name="mid", bufs=2) as mpool,
            tc.tile_pool(name="trig", bufs=2) as tgpool,
            tc.tile_pool(name="outp", bufs=2) as opool,
            tc.tile_pool(name="tccp", bufs=2) as tccpool,
        ):
            # GPSIMD library for local_scatter; must precede every
            # library-tracked pool instruction (memset is built-in).
            nc.gpsimd.load_library(library_config.local_scatter)

            halfpi = cpool.tile([B, 1], F32, tag="halfpi")
            nc.gpsimd.memset(halfpi[:], HALF_PI)
            zbias = cpool.tile([B, 1], F32, tag="zbias")
            nc.gpsimd.memset(zbias[:], 0.0)

            # --- coefficients: A = 0.5*cos(theta), Bc = -0.5*sin(theta) ---
            th = cpool.tile([B, NCLS], F32, tag="th")
            nc.scalar.dma_start(out=th[:], in_=td[:].to_broadcast((B, NCLS)))

            # Range reduction with standard ALUs (valid for |x| < 3pi):
            #   y = x - 2pi*(x > pi) + 2pi*(x < -pi)  in [-pi, pi]
            #   sin(x) = Sin(y);  cos(x) = cos(|y|) = Sin(-|y| + pi/2)
            def wrap2(dst_y, src, g, l, y1):
                nc.vector.tensor_scalar(g, src, PI, None, ALU.is_gt)
                nc.vector.tensor_scalar(l, src, -PI, None, ALU.is_lt)
                nc.vector.scalar_tensor_tensor(y1, g, -TWO_PI, src, ALU.mult, ALU.add)
                nc.vector.scalar_tensor_tensor(dst_y, l, TWO_PI, y1, ALU.mult, ALU.add)

            thy = cpool.tile([B, NCLS], F32, tag="thy")
            thz = cpool.tile([B, NCLS], F32, tag="thz")
            tg = cpool.tile([B, NCLS], F32, tag="tg")
            tl = cpool.tile([B, NCLS], F32, tag="tl")
            t1 = cpool.tile([B, NCLS], F32, tag="t1")
            wrap2(thy[:], th[:], tg[:], tl[:], t1[:])
            nc.scalar.activation(thz[:], thy[:], Abs, bias=zbias[:, 0:1])
            A = cpool.tile([B, NCLS], F32, tag="A")
            Bc = cpool.tile([B, NCLS], F32, tag="Bc")
            nc.scalar.activation(A[:], thz[:], Sin, bias=halfpi[:, 0:1], scale=-1.0)
            nc.scalar.activation(Bc[:], thy[:], Sin, bias=zbias[:, 0:1])
            nc.vector.tensor_scalar_mul(A[:], A[:], 0.5)
            nc.vector.tensor_scalar_mul(Bc[:], Bc[:], -0.5)

            # --- inputs ---
            # x cast to fp16 during the DMA itself (SWDGE converts in the
            # SDMA datapath; no engine time).
            xh = ipool.tile([B, NPIX], F16, tag="xh")
            nc.gpsimd.dma_start(out=xh[:], in_=xd[:])
            invt = ipool.tile([B, NPIX], I16, tag="invt")
            nc.scalar.dma_start(out=invt[:], in_=invd[:])
            c1t = ipool.tile([B, NIDX + NCC], I16, tag="c1t")
            nc.scalar.dma_start(out=c1t[:], in_=c1d[:])
            pct = ipool.tile([B, NIDX], I16, tag="pct")
            nc.scalar.dma_start(out=pct[:], in_=pcd[:])

            for rep in range(n_repeat):
                if "s" not in parts:
                    continue
                # --- scatter-gather: acc[b, j] = x[b, pair_idx[b, j]] ---
                # ds = [d0 (first occurrences) | replicated comp slots];
                # one fused chain round then covers every rank >= 1.
                ds = gpool.tile([B, NIDX + NCC], F16, tag="ds")
                d0 = ds[:, 0:NIDX]
                nc.gpsimd.local_scatter(
                    d0, xh[:], invt[:], channels=B, num_elems=NIDX, num_idxs=NPIX
                )
                if n_ranks > 2:
                    comp = gpool.tile([B, NCOMP], F16, tag="comp")
                    nc.gpsimd.local_scatter(
                        comp[:], d0, pct[:],
                        channels=B, num_elems=NCOMP, num_idxs=NIDX,
                    )
                    # replicate on GPSIMD itself: keeps the scatter chain
                    # in one engine queue (no DVE round trip mid-chain)
                    for t in range(n_ranks - 2):
                        nc.gpsimd.tensor_copy(
                            ds[:, NIDX + t * NCOMP : NIDX + (t + 1) * NCOMP], comp[:]
                        )
                dx = gpool.tile([B, NIDX], F16, tag="dx")
                nc.gpsimd.local_scatter(
                    dx[:], ds[:], c1t[:],
                    channels=B, num_elems=NIDX, num_idxs=NIDX + NCC,
                )
                acc = gpool.tile([B, NIDX], F16, tag="acch")
                nc.vector.tensor_add(acc[:], d0, dx[:])

                if "t" not in parts:
                    continue
                # acc is laid out de-interleaved by the host's slot map:
                # [a_0..a_459 | b_0..b_459] — every op below is step-1.
                # All trig in fp16: full-width, few instructions (ACT has a
                # ~224-cycle and DVE a ~58-cycle fixed cost per op).
                trig = {}
                for half, name in ((0, "a"), (1, "b")):
                    src = acc[:, half * NPAIR : (half + 1) * NPAIR]
                    g = mpool.tile([B, NPAIR], F16, tag=f"g{name}")
                    l = mpool.tile([B, NPAIR], F16, tag=f"l{name}")
                    y1 = mpool.tile([B, NPAIR], F16, tag=f"y1{name}")
                    y = mpool.tile([B, NPAIR], F16, tag=f"y{name}")
                    wrap2(y[:], src, g[:], l[:], y1[:])
                    ya = mpool.tile([B, NPAIR], F16, tag=f"ya{name}")
                    nc.scalar.activation(ya[:], y[:], Abs, bias=zbias[:, 0:1])
                    c = tgpool.tile([B, NPAIR], F16, tag=f"c{name}")
                    s = tgpool.tile([B, NPAIR], F16, tag=f"s{name}")
                    nc.scalar.activation(
                        c[:], ya[:], Sin, bias=halfpi[:, 0:1], scale=-1.0
                    )
                    nc.scalar.activation(s[:], y[:], Sin, bias=zbias[:, 0:1])
                    trig[name] = (c, s)
                ca, sa = trig["a"]
                cb, sb = trig["b"]

                # products at full width
                v = tgpool.tile([B, NPAIR], F16, tag="v")
                wv = tgpool.tile([B, NPAIR], F16, tag="wv")
                nc.vector.tensor_mul(v[:], sa[:], sb[:])
                nc.vector.tensor_mul(wv[:], ca[:], cb[:])

                if "c" not in parts:
                    continue
                # per-class tcc (ACT) and even-column stt (DVE) interleaved
                # so the two engines ping-pong instead of serializing in
                # two monolithic blocks; odd columns + DMA per span after.
                PS = NPAIR // n_spans
                obs = []
                for s0 in range(n_spans):
                    ob = opool.tile([B, PS * 2 * NCLS], F32, tag=f"ob{s0}")
                    obs.append(ob)
                for c in range(NCLS):
                    tcc = tccpool.tile([B, NPAIR], F16, tag=f"tcc{c}")
                    nc.scalar.activation(
                        tcc[:], v[:], Copy, bias=0.5, scale=Bc[:, c : c + 1]
                    )
                    for s0 in range(n_spans):
                        sl = slice(s0 * PS, (s0 + 1) * PS)
                        nc.vector.scalar_tensor_tensor(
                            obs[s0][:, c : PS * 2 * NCLS : 2 * NCLS],
                            ca[:, sl],
                            A[:, c : c + 1],
                            tcc[:, sl],
                            ALU.mult,
                            ALU.add,
                        )
                for s0 in range(n_spans):
                    ob3 = obs[s0][:].rearrange("p (t k) -> p t k", k=2 * NCLS)
                    sl = slice(s0 * PS, (s0 + 1) * PS)
                    nc.scalar.activation(
                        ob3[:, :, NCLS : 2 * NCLS],
                        wv[:, sl, None].broadcast_to((B, PS, NCLS)),
                        Copy,
                        bias=0.5,
                        scale=0.5,
                    )
                    nc.sync.dma_start(
                        out=od[:, s0 * PS * 2 * NCLS : (s0 + 1) * PS * 2 * NCLS],
                        in_=obs[s0][:],
                    )
    _legalize_sync_waits(nc)
    mybir.codegen_inst_isa_subclasses(nc)
    return nc


def _index_prep(pidx):
    """pair_idx [B_FULL, NIDX] (int, < NPIX) -> inv, c1cc, pc, n_ranks.

    Pure index re-encoding (host touches no model data): the gather
    vals[b, j] = x[b, pidx[b, j]] becomes device scatters
      d0[inv[b,pix]] = x[b,pix]; dx[c1cc[b,*]] covers ranks >= 1 via
      first-occurrence slots and 64 compacted slots (pc) for ranks >= 2.

    Destination slots use the de-interleaved map sigma(j) =
    (j%2)*460 + j//2, so a-angles land in acc[:, 0:460] and b-angles in
    acc[:, 460:920] — every downstream engine op is then stride-1.
    """
    R, N = pidx.shape
    sigma = (np.arange(N, dtype=np.int16) % 2) * (N // 2) + np.arange(
        N, dtype=np.int16
    ) // 2
    flat = pidx.astype(np.int64)
    keys = (flat + NPIX * np.arange(R, dtype=np.int64)[:, None]).ravel()
    order = np.argsort(keys, kind="stable")
    sk = keys[order]
    pos = np.arange(R * N)
    first = np.r_[True, sk[1:] != sk[:-1]]
    grp_start = np.maximum.accumulate(np.where(first, pos, 0))
    rank = pos - grp_start
    row = order // N
    j = sigma[order % N]  # dst slots in de-interleaved layout
    firstj = j[grp_start]
    n_ranks = int(rank.max()) + 1

    inv = np.full((R, NPIX), -1, np.int16)
    m0 = rank == 0
    inv[row[m0], flat.ravel()[order[m0]]] = j[m0]

    c1 = np.full((R, N), -1, np.int16)
    m1 = rank == 1
    c1[row[m1], firstj[m1]] = j[m1]

    # compaction slots for pixels used >= 3 times (their rank-2 entry)
    pc = np.full((R, N), -1, np.int16)
    NCC = max(n_ranks - 2, 0) * NCOMP
    cc = np.full((R, NCC), -1, np.int16)
    m2 = rank == 2
    rows2 = row[m2]
    rfirst = np.r_[True, rows2[1:] != rows2[:-1]] if rows2.size else np.array([], bool)
    rstart = (
        np.maximum.accumulate(np.where(rfirst, np.arange(rows2.size), 0))
        if rows2.size
        else np.array([], np.int64)
    )
    slot = (np.arange(rows2.size) - rstart).astype(np.int16)
    assert slot.size == 0 or slot.max() < NCOMP, "NCOMP overflow"
    pc[rows2, firstj[m2]] = slot
    # slot lookup per group for ranks >= 2
    slot_of_group = {}
    g2 = grp_start[m2]
    for gg, ss, rr in zip(g2, slot, rows2):
        slot_of_group[gg] = ss
    for k in range(2, n_ranks):
        mk = rank == k
        gk = grp_start[mk]
        sk_ = np.array([slot_of_group[g] for g in gk], dtype=np.int64)
        cc[row[mk], (k - 2) * NCOMP + sk_] = j[mk]
    c1cc = np.ascontiguousarray(np.concatenate([c1, cc], axis=1))
    return inv, c1cc, pc, n_ranks


def _prep_inputs(x, theta, pair_idx):
    """Full inputs -> list of per-core input maps (host-side sharding and
    index re-encoding only; model data x/theta untouched beyond reshape)."""
    x = np.ascontiguousarray(np.asarray(x, dtype=np.float32).reshape(B_FULL, NPIX))
    theta = np.ascontiguousarray(np.asarray(theta, dtype=np.float32).reshape(1, NCLS))
    pidx = np.asarray(pair_idx).reshape(B_FULL, NIDX)
    inv, c1cc, pc, n_ranks = _index_prep(pidx)
    in_maps = []
    for k in range(N_CORES):
        sl = slice(k * B, (k + 1) * B)
        in_maps.append(
            {
                "x": x[sl],
                "inv": inv[sl],
                "c1cc": c1cc[sl],
                "pc": pc[sl],
                "theta": theta,
            }
        )
    return in_maps, n_ranks


_CACHED = {}


def kernel(x, theta, pair_idx):
    from concourse.bass_utils import run_bass_kernel_spmd

    in_maps, n_ranks = _prep_inputs(x, theta, pair_idx)
    if ("nc", n_ranks) not in _CACHED:
        _CACHED[("nc", n_ranks)] = build_kernel(n_ranks=n_ranks)
    nc = _CACHED[("nc", n_ranks)]
    res = run_bass_kernel_spmd(nc, in_maps, core_ids=list(range(N_CORES)))
    out = np.concatenate([r["out"] for r in res.results], axis=0)
    return out.reshape(B_FULL, NIDX, NCLS)
